# revision 1
# baseline (speedup 1.0000x reference)
"""ARAP loss kernel for Trainium2 (8 NeuronCores, Bass/Tile).

Computes mean(|sum((x[d]-x[s])^2) - sum((dx[d]-dx[s])^2)|) over a directed
edge list. The edge set is symmetric ((s,d) present iff (d,s) present) and the
per-edge value is symmetric in (s,d), so we process only pairs with s < d and
double the sum (self-loops contribute exactly 0). Edges are sharded across the
8 cores; the packed vertex table [NV, 8] f32 (x | dx | pad) is replicated.
Per-core, tiles of 128 edges fetch both endpoint rows with indirect DMA
(128-row gathers), the DVE computes the per-edge |diffx - diffdx| and reduces
into a per-partition accumulator; the host sums the 8x128 partials and divides
by the full edge count.
"""
import math
import os
import sys
import tempfile

sys.path.insert(0, "/opt/trn_rl_repo")

import numpy as np

import concourse.bacc as bacc
import concourse.bass as bass
import concourse.mybir as mybir
from concourse import tile
from concourse.bass import ds
from concourse.bass_utils import run_bass_kernel_spmd

N_CORES = 8
U = 16                # groups of 128 edges per loop body
EPB = 128 * U         # edges per loop body iteration

LAST_EXEC_NS = None

_cache = {}


def _install_trace_shim():
    """Provide antenv.axon_hooks so trace=True works under axon."""
    import contextlib
    import ctypes
    import types

    if "antenv.axon_hooks" in sys.modules:
        return
    try:
        lib = ctypes.CDLL("/opt/axon/libaxon_pjrt.so")
    except OSError:
        return
    if not hasattr(lib, "axon_start_nrt_profile"):
        return
    lib.axon_start_nrt_profile.argtypes = [ctypes.POINTER(ctypes.c_int64), ctypes.c_size_t]
    lib.axon_start_nrt_profile.restype = ctypes.c_int64
    lib.axon_stop_nrt_profile.argtypes = [ctypes.c_char_p]
    lib.axon_stop_nrt_profile.restype = ctypes.c_int64

    @contextlib.contextmanager
    def _hook(output_dir, device_ids):
        import jax

        jax.devices()
        if device_ids:
            ids = (ctypes.c_int64 * len(device_ids))(*device_ids)
            rc = lib.axon_start_nrt_profile(ids, len(device_ids))
        else:
            rc = lib.axon_start_nrt_profile(None, 0)
        if rc != 0:
            raise RuntimeError(f"axon_start_nrt_profile rc={rc}")
        try:
            yield
        finally:
            n = lib.axon_stop_nrt_profile(str(output_dir).encode())
            print(f"profile: {n} file(s) written to {output_dir}", file=sys.stderr)

    mod = types.ModuleType("antenv.axon_hooks")
    mod.get_axon_ntff_profile_hook = lambda: _hook
    mod.set_axon_ntff_profile_hook = lambda h: None
    sys.modules["antenv.axon_hooks"] = mod


def _build(nv, nt):
    """Build the SPMD bass program for nt body-iterations per core."""
    key = (nv, nt)
    if key in _cache:
        return _cache[key]

    f32 = mybir.dt.float32
    i32 = mybir.dt.int32

    nc = bacc.Bacc(None, target_bir_lowering=False)
    vt_d = nc.dram_tensor("vt", [nv, 8], f32, kind="ExternalInput")
    eidx_d = nc.dram_tensor("eidx", [nt * 128, 2 * U], i32, kind="ExternalInput")
    out_d = nc.dram_tensor("out", [128, 1], f32, kind="ExternalOutput")

    with tile.TileContext(nc) as tc:
        with (
            tc.tile_pool(name="accp", bufs=1) as accp,
            tc.tile_pool(name="iop", bufs=3) as iop,
            tc.tile_pool(name="gp", bufs=3) as gp,
            tc.tile_pool(name="cp", bufs=3) as cp,
        ):
            acc = accp.tile([128, 1], f32)
            nc.vector.memset(acc[:], 0.0)
            with tc.For_i(0, nt * 128, 128) as t:
                it = iop.tile([128, 2 * U], i32, tag="idx")
                nc.sync.dma_start(it[:], eidx_d[ds(t, 128), :])
                gs = gp.tile([128, U * 8], f32, tag="gs")
                gd = gp.tile([128, U * 8], f32, tag="gd")
                for j in range(U):
                    nc.gpsimd.indirect_dma_start(
                        out=gs[:, j * 8:(j + 1) * 8],
                        out_offset=None,
                        in_=vt_d[:],
                        in_offset=bass.IndirectOffsetOnAxis(ap=it[:, j:j + 1], axis=0),
                    )
                    nc.gpsimd.indirect_dma_start(
                        out=gd[:, j * 8:(j + 1) * 8],
                        out_offset=None,
                        in_=vt_d[:],
                        in_offset=bass.IndirectOffsetOnAxis(ap=it[:, U + j:U + j + 1], axis=0),
                    )
                dif = cp.tile([128, U * 8], f32, tag="dif")
                nc.vector.tensor_sub(dif[:], gd[:], gs[:])
                sq = cp.tile([128, U * 8], f32, tag="sq")
                nc.vector.tensor_mul(sq[:], dif[:], dif[:])
                sq3 = sq[:].rearrange("p (u c) -> p u c", c=8)
                sx = cp.tile([128, U], f32, tag="sx")
                sdx = cp.tile([128, U], f32, tag="sdx")
                nc.vector.tensor_reduce(
                    sx[:].rearrange("p (u c) -> p u c", c=1),
                    sq3[:, :, 0:3], axis=mybir.AxisListType.X, op=mybir.AluOpType.add,
                )
                nc.vector.tensor_reduce(
                    sdx[:].rearrange("p (u c) -> p u c", c=1),
                    sq3[:, :, 3:6], axis=mybir.AxisListType.X, op=mybir.AluOpType.add,
                )
                s = cp.tile([128, U], f32, tag="s")
                nc.vector.tensor_sub(s[:], sx[:], sdx[:])
                contrib = cp.tile([128, 1], f32, tag="contrib")
                nc.vector.tensor_reduce(
                    contrib[:], s[:], axis=mybir.AxisListType.X,
                    op=mybir.AluOpType.add, apply_absolute_value=True,
                )
                nc.vector.tensor_add(acc[:], acc[:], contrib[:])
            nc.sync.dma_start(out_d[:], acc[:])
    nc.compile()
    _cache[key] = nc
    return nc


def kernel(dx, x, edge_src, edge_dst):
    global LAST_EXEC_NS
    x = np.asarray(x, dtype=np.float32)
    dx = np.asarray(dx, dtype=np.float32)
    s = np.asarray(edge_src).astype(np.int64)
    d = np.asarray(edge_dst).astype(np.int64)
    nv = x.shape[0]
    e_total = s.shape[0]

    # packed vertex table (pure layout, replicated on every core)
    vt = np.zeros((nv, 8), dtype=np.float32)
    vt[:, 0:3] = x
    vt[:, 3:6] = dx

    # keep one direction of each symmetric pair; self loops contribute 0
    mask = s < d
    ws = s[mask].astype(np.int32)
    wd = d[mask].astype(np.int32)
    w = ws.shape[0]

    per_core = math.ceil(w / N_CORES)
    nt = max(1, math.ceil(per_core / EPB))
    slots = nt * EPB

    in_maps = []
    for c in range(N_CORES):
        lo = min(c * per_core, w)
        hi = min(lo + per_core, w)
        cs = np.zeros(slots, dtype=np.int32)
        cdst = np.zeros(slots, dtype=np.int32)
        cs[: hi - lo] = ws[lo:hi]
        cdst[: hi - lo] = wd[lo:hi]
        # edge (t, j, p) -> eidx[t*128 + p, j] (src) / [.., U + j] (dst)
        eidx = np.empty((nt, 128, 2 * U), dtype=np.int32)
        eidx[:, :, :U] = cs.reshape(nt, U, 128).transpose(0, 2, 1)
        eidx[:, :, U:] = cdst.reshape(nt, U, 128).transpose(0, 2, 1)
        in_maps.append({"vt": vt, "eidx": eidx.reshape(nt * 128, 2 * U)})

    trace = os.environ.get("ARAP_TRACE") == "1"
    if trace:
        _install_trace_shim()

    nc = _build(nv, nt)
    res = run_bass_kernel_spmd(
        nc, in_maps, core_ids=list(range(N_CORES)), trace=trace,
        tmpdir=tempfile.mkdtemp(prefix="arap_") if trace else None,
    )
    if trace:
        LAST_EXEC_NS = res.exec_time_ns

    total = 0.0
    for c in range(N_CORES):
        total += res.results[c]["out"].astype(np.float64).sum()
    return np.asarray(2.0 * total / e_total, dtype=np.float32)



# revision 3
# speedup vs baseline: 18.3012x; 18.3012x over previous
"""ARAP loss kernel for Trainium2 (8 NeuronCores, Bass/Tile).

Computes mean(|sum((x[d]-x[s])^2) - sum((dx[d]-dx[s])^2)|) over a directed
edge list. The edge set is symmetric ((s,d) present iff (d,s) present) and the
per-edge value is symmetric in (s,d), so we process only pairs with s < d and
double the sum (self-loops contribute exactly 0). Edges are sharded across the
8 cores; the packed vertex table [NV, 8] f32 (x | dx | pad) is replicated.
Per-core, tiles of 128 edges fetch both endpoint rows with indirect DMA
(128-row gathers), the DVE computes the per-edge |diffx - diffdx| and reduces
into a per-partition accumulator; the host sums the 8x128 partials and divides
by the full edge count.
"""
import math
import os
import sys
import tempfile

sys.path.insert(0, "/opt/trn_rl_repo")

import numpy as np

import concourse.bacc as bacc
import concourse.bass as bass
import concourse.mybir as mybir
from concourse import tile
from concourse.bass import ds
from concourse.bass_utils import run_bass_kernel_spmd

N_CORES = 8
U = 64                # groups of 128 edges per loop body
EPB = 128 * U         # edges per loop body iteration

LAST_EXEC_NS = None

_cache = {}


def _install_trace_shim():
    """Provide antenv.axon_hooks so trace=True works under axon."""
    import contextlib
    import ctypes
    import types

    if "antenv.axon_hooks" in sys.modules:
        return
    try:
        lib = ctypes.CDLL("/opt/axon/libaxon_pjrt.so")
    except OSError:
        return
    if not hasattr(lib, "axon_start_nrt_profile"):
        return
    lib.axon_start_nrt_profile.argtypes = [ctypes.POINTER(ctypes.c_int64), ctypes.c_size_t]
    lib.axon_start_nrt_profile.restype = ctypes.c_int64
    lib.axon_stop_nrt_profile.argtypes = [ctypes.c_char_p]
    lib.axon_stop_nrt_profile.restype = ctypes.c_int64

    @contextlib.contextmanager
    def _hook(output_dir, device_ids):
        import jax

        jax.devices()
        if device_ids:
            ids = (ctypes.c_int64 * len(device_ids))(*device_ids)
            rc = lib.axon_start_nrt_profile(ids, len(device_ids))
        else:
            rc = lib.axon_start_nrt_profile(None, 0)
        if rc != 0:
            raise RuntimeError(f"axon_start_nrt_profile rc={rc}")
        try:
            yield
        finally:
            n = lib.axon_stop_nrt_profile(str(output_dir).encode())
            print(f"profile: {n} file(s) written to {output_dir}", file=sys.stderr)

    mod = types.ModuleType("antenv.axon_hooks")
    mod.get_axon_ntff_profile_hook = lambda: _hook
    mod.set_axon_ntff_profile_hook = lambda h: None
    sys.modules["antenv.axon_hooks"] = mod


def _build(nv, nt):
    """Build the SPMD bass program for nt body-iterations per core."""
    key = (nv, nt)
    if key in _cache:
        return _cache[key]

    f32 = mybir.dt.float32
    i32 = mybir.dt.int32

    nc = bacc.Bacc(None, target_bir_lowering=False)
    vt_d = nc.dram_tensor("vt", [nv, 8], f32, kind="ExternalInput")
    eidx_d = nc.dram_tensor("eidx", [nt * 128, 2 * U], i32, kind="ExternalInput")
    out_d = nc.dram_tensor("out", [128, 1], f32, kind="ExternalOutput")

    with tile.TileContext(nc) as tc:
        with (
            tc.tile_pool(name="accp", bufs=1) as accp,
            tc.tile_pool(name="iop", bufs=3) as iop,
            tc.tile_pool(name="gp", bufs=3) as gp,
            tc.tile_pool(name="cp", bufs=3) as cp,
        ):
            acc = accp.tile([128, 1], f32)
            nc.vector.memset(acc[:], 0.0)
            with tc.For_i(0, nt * 128, 128) as t:
                it = iop.tile([128, 2 * U], i32, tag="idx")
                nc.sync.dma_start(it[:], eidx_d[ds(t, 128), :])
                # one SWDGE instruction gathers all 2*U rows per partition:
                # g[p, j*8:(j+1)*8] = vt[it[p, j]]  (src cols 0..U-1, dst U..2U-1)
                g = gp.tile([128, 2 * U * 8], f32, tag="g")
                nc.gpsimd.indirect_dma_start(
                    out=g[:],
                    out_offset=None,
                    in_=vt_d[:],
                    in_offset=bass.IndirectOffsetOnAxis(ap=it[:], axis=0),
                )
                dif = cp.tile([128, U * 8], f32, tag="dif")
                nc.vector.tensor_sub(dif[:], g[:, U * 8:2 * U * 8], g[:, 0:U * 8])
                sq = cp.tile([128, U * 8], f32, tag="sq")
                nc.vector.tensor_mul(sq[:], dif[:], dif[:])
                sq3 = sq[:].rearrange("p (u c) -> p u c", c=8)
                sx = cp.tile([128, U], f32, tag="sx")
                sdx = cp.tile([128, U], f32, tag="sdx")
                nc.vector.tensor_reduce(
                    sx[:].rearrange("p (u c) -> p u c", c=1),
                    sq3[:, :, 0:3], axis=mybir.AxisListType.X, op=mybir.AluOpType.add,
                )
                nc.vector.tensor_reduce(
                    sdx[:].rearrange("p (u c) -> p u c", c=1),
                    sq3[:, :, 3:6], axis=mybir.AxisListType.X, op=mybir.AluOpType.add,
                )
                s = cp.tile([128, U], f32, tag="s")
                nc.vector.tensor_sub(s[:], sx[:], sdx[:])
                contrib = cp.tile([128, 1], f32, tag="contrib")
                nc.vector.tensor_reduce(
                    contrib[:], s[:], axis=mybir.AxisListType.X,
                    op=mybir.AluOpType.add, apply_absolute_value=True,
                )
                nc.vector.tensor_add(acc[:], acc[:], contrib[:])
            nc.sync.dma_start(out_d[:], acc[:])
    nc.compile()
    _cache[key] = nc
    return nc


def kernel(dx, x, edge_src, edge_dst):
    global LAST_EXEC_NS
    x = np.asarray(x, dtype=np.float32)
    dx = np.asarray(dx, dtype=np.float32)
    s = np.asarray(edge_src).astype(np.int64)
    d = np.asarray(edge_dst).astype(np.int64)
    nv = x.shape[0]
    e_total = s.shape[0]

    # packed vertex table (pure layout, replicated on every core)
    vt = np.zeros((nv, 8), dtype=np.float32)
    vt[:, 0:3] = x
    vt[:, 3:6] = dx

    # keep one direction of each symmetric pair; self loops contribute 0
    mask = s < d
    ws = s[mask].astype(np.int32)
    wd = d[mask].astype(np.int32)
    w = ws.shape[0]

    per_core = math.ceil(w / N_CORES)
    nt = max(1, math.ceil(per_core / EPB))
    slots = nt * EPB

    in_maps = []
    for c in range(N_CORES):
        lo = min(c * per_core, w)
        hi = min(lo + per_core, w)
        cs = np.zeros(slots, dtype=np.int32)
        cdst = np.zeros(slots, dtype=np.int32)
        cs[: hi - lo] = ws[lo:hi]
        cdst[: hi - lo] = wd[lo:hi]
        # edge (t, j, p) -> eidx[t*128 + p, j] (src) / [.., U + j] (dst)
        eidx = np.empty((nt, 128, 2 * U), dtype=np.int32)
        eidx[:, :, :U] = cs.reshape(nt, U, 128).transpose(0, 2, 1)
        eidx[:, :, U:] = cdst.reshape(nt, U, 128).transpose(0, 2, 1)
        in_maps.append({"vt": vt, "eidx": eidx.reshape(nt * 128, 2 * U)})

    trace = os.environ.get("ARAP_TRACE") == "1"
    if trace:
        _install_trace_shim()

    nc = _build(nv, nt)
    res = run_bass_kernel_spmd(
        nc, in_maps, core_ids=list(range(N_CORES)), trace=trace,
        tmpdir=tempfile.mkdtemp(prefix="arap_") if trace else None,
    )
    if trace:
        LAST_EXEC_NS = res.exec_time_ns

    total = 0.0
    for c in range(N_CORES):
        total += res.results[c]["out"].astype(np.float64).sum()
    return np.asarray(2.0 * total / e_total, dtype=np.float32)

